# revision 37
# baseline (speedup 1.0000x reference)
"""Multi-head attention (B=1, S=4096, D=768, H=12, Hd=64) on 8 trn2 cores.

Sharding: 2 head-groups (6 heads = 384 dims, Megatron column-split wq/wk/wv,
row-split ww) x 4 query-chunks (1024 rows).  core = g*4 + c.
Each core returns a partial output [1024, 768]; host sums the 2 group
partials per chunk and adds (bv @ ww.T + bw).

v2 design (ACT-bound fused schedule):
  - All projections / scores in bf16 (1 cyc/row on PE, exact-enough).
  - attnV in fp8e4 DoubleRow (2 key-tiles per instruction): V8 holds
    fp8(64*V) rows per key with a ones column at 64 and 63 pad cols (dual-fp8
    ldweights requires the full 128 weight columns); pt = fp8(8*exp(s/8))
    written directly by the ACT exp.  Scale bookkeeping: numerator rows are
    512*(P.V), denominator row 64 is 8*sum(P) -> y6 = 64*out; ww is
    pre-divided by 64 on the host.
  - The exp stream on ACT (164us of columns) is the binding engine.  The
    key axis is split in NSPLIT=4 quarters; within each quarter the 12
    rounds (qh, p, head) run scores->exp->attnV pipelined through a
    double-buffered [128,3,512] psum pair, while K/V projection blocks
    (quarters 0-2) and the out-projection (quarter 3) execute in the PE
    gaps as interleaved "filler" pieces.  attnV accumulates per-quarter in
    a single psum bank and spills/accumulates into acc (SBUF, f32).
  - psum budget: scores 2x3 banks + o 1 + filler 1 = 8.
"""

import sys

if "/opt/trn_rl_repo" not in sys.path:
    sys.path.insert(0, "/opt/trn_rl_repo")

import math
from collections import deque

import numpy as np
import ml_dtypes

import concourse.bacc as bacc
import concourse.bass as bass
import concourse.mybir as mybir
import concourse.tile as tile
from concourse.bass_utils import run_bass_kernel_spmd
from concourse.vector_clock import ScopedClock

F32 = mybir.dt.float32
BF = mybir.dt.bfloat16
F8 = mybir.dt.float8e4
AF = mybir.ActivationFunctionType
DR = mybir.MatmulPerfMode.DoubleRow

S = 4096          # sequence length
D = 768           # model dim
NG = 2            # head groups (cores axis 1)
NC = 4            # query chunks (cores axis 2)
DH = D // NG      # dims per group = 384
NP = DH // 128    # head pairs per group = 3
NH = 2 * NP       # heads per group = 6
SQ = S // NC      # queries per core = 1024
KO = D // 128     # contraction subtiles = 6
NJ = S // 128     # key tiles = 32
SCALE = 0.125     # 1/sqrt(64)
LN8 = float(math.log(8.0))
VSCALE = 32.0     # host scale folded into wv (and 1/VSCALE into ww);
                  # max |VSCALE*v| ~ 127 stays below the TRN e4m3 max of 240
                  # (the DVE f32->fp8 conversion overflows instead of
                  # saturating, so headroom is required)

import os
PROBE = os.environ.get("PROBE", "")   # timing probes: halfexp / halfscores

NSPLIT = 4        # key-axis quarters
JQ = NJ // NSPLIT           # j-tiles per quarter = 8
NPAIR = JQ // 2             # DoubleRow pairs per round-quarter = 4
if os.environ.get("CS_CFG", "2") == "2":
    QCHUNKS = [2, 2, 2, 2]  # exp chunk sizes covering JQ j-tiles
    SC_BUFS = 3             # psum: sc 3x2 banks + o + kv = 8
else:
    QCHUNKS = [3, 3, 2]
    SC_BUFS = 2             # psum: sc 2x3 banks + o + kv = 8
SCW = max(QCHUNKS)
ROUNDS = [(qh, p, h) for qh in range(2) for p in range(NP) for h in range(2)]

_PATCHED = False


def _patch_drain():
    """walrus in this container rejects >1 sync-wait per instruction
    ("Too many sync wait commands").  TileContext's tail drain aggregates one
    wait per live tile semaphore; redistribute them one-per-nop.  (Bacc's
    generate_event_semaphores handles the rest of the kernel.)"""
    global _PATCHED
    if _PATCHED:
        return
    _PATCHED = True

    def _drain_and_barrier(self, tick_clock, wait_clock):
        nc = self.nc
        drain_inst = nc.sync.drain()
        wait_clock.add_sem_waits(
            drain_inst.ins, ScopedClock({None: tick_clock.global_clock})
        )
        si = drain_inst.ins.sync_info
        waits = list(si.on_wait) if si is not None else []
        if len(waits) > 1:
            drain_inst.ins.sync_info = mybir.SyncInfo(
                on_wait=[waits[0]], on_update=list(si.on_update)
            )
            for w in waits[1:]:
                nop = nc.sync.nop(nofuse=True)
                nop.ins.sync_info = mybir.SyncInfo(on_wait=[w], on_update=[])
        nc.all_engine_barrier()
        assert self.sems is not None
        popped = nc._tile_sem_poison_stack.pop()
        assert popped is self._sem_poison
        nc.clear_and_free_semaphores(list(self.sems.allocated().values()))
        nc.all_engine_barrier()

    tile.TileContext._drain_and_barrier = _drain_and_barrier


def build_nc(loop_n=None, debug=False):
    _patch_drain()
    nc = bacc.Bacc("TRN2", target_bir_lowering=False)

    xT = nc.dram_tensor("xT", [D, S], BF, kind="ExternalInput")
    xqT = nc.dram_tensor("xqT", [D, SQ], BF, kind="ExternalInput")
    wqT = nc.dram_tensor("wqT", [D, DH], BF, kind="ExternalInput")
    wkT = nc.dram_tensor("wkT", [D, DH], BF, kind="ExternalInput")
    wvT = nc.dram_tensor("wvT", [D, DH], BF, kind="ExternalInput")  # x VSCALE
    wwT = nc.dram_tensor("wwT", [DH, D], BF, kind="ExternalInput")  # / VSCALE
    bq = nc.dram_tensor("bq", [128, NP], F32, kind="ExternalInput")
    bk = nc.dram_tensor("bk", [128, NP], F32, kind="ExternalInput")
    out = nc.dram_tensor("out", [SQ, D], F32, kind="ExternalOutput")

    xT_r = xT.rearrange("(ko p) n -> p ko n", p=128)
    xqT_r = xqT.rearrange("(ko p) n -> p ko n", p=128)
    wqT_r = wqT.rearrange("(ko p) m -> p ko m", p=128)
    wkT_r = wkT.rearrange("(ko p) m -> p ko m", p=128)
    wvT_r = wvT.rearrange("(ko p) m -> p ko m", p=128)
    ww6_r = wwT.rearrange("(h l) o -> l h o", l=64)   # [64, 6, 768]

    with tile.TileContext(nc) as tc:
        import contextlib

        with contextlib.ExitStack() as ctx:
            if loop_n is not None:
                ctx.enter_context(tc.For_i(0, loop_n, 1))
            persist = ctx.enter_context(tc.tile_pool(name="persist", bufs=1))
            KT = persist.tile([128, NP, S], BF)         # 24KB/part
            QTz = persist.tile([128, NH, SQ], BF)       # 12KB/part
            V8 = persist.tile([128, NH, NJ, 128], F8)   # 24KB/part
            acc = persist.tile([128, 12, 512], F32)     # 24KB/part
            ptr = persist.tile([128, 2, 6, 512], F8)    # exp rings, 6KB/part
            y6 = persist.tile([128, NH, SQ], BF)        # 12KB/part
            out_acc = persist.tile([128, 8, D], F32)    # 24KB/part
            ww6 = persist.tile([128, NH, D], BF)        # 9KB/part
            lnb = persist.tile([128, 1], F32)
            bq_sb = persist.tile([128, NP], F32)
            bk_sb = persist.tile([128, NP], F32)

            w_pool = ctx.enter_context(tc.tile_pool(name="w", bufs=1))
            wk_sb = w_pool.tile([128, KO, DH], BF)
            wv_sb = w_pool.tile([128, KO, DH], BF)
            wq_sb = w_pool.tile([128, KO, DH], BF)

            xs = ctx.enter_context(tc.tile_pool(name="xs", bufs=3))
            ob_pool = ctx.enter_context(tc.tile_pool(name="ob", bufs=2))
            bc_pool = ctx.enter_context(tc.tile_pool(name="bc", bufs=2))

            sc_pool = ctx.enter_context(
                tc.tile_pool(name="sc", bufs=SC_BUFS, space="PSUM"))
            o_pool = ctx.enter_context(
                tc.tile_pool(name="o", bufs=1, space="PSUM"))
            kv_pool = ctx.enter_context(
                tc.tile_pool(name="kv", bufs=1, space="PSUM"))

            # ---------------- init + weight DMA ----------------
            # DMA order matters: block-0 x data and wk must land first.
            nc.sync.dma_start(wk_sb[:], wkT_r[:])
            nc.sync.dma_start(bk_sb[:], bk[:])
            nc.vector.memset(lnb[:], LN8)
            # zero inits run on gpsimd (idle early) so the DVE queue stays
            # clear for the K bias-adds of the first blocks
            for h in range(NH):
                if h % 2 == 0:
                    nc.gpsimd.memset(QTz[64:128, h, :], 0.0)
                else:
                    nc.gpsimd.memset(QTz[0:64, h, :], 0.0)
            # upper halves zero: out-proj runs K=128 over head pairs' rows
            nc.gpsimd.memset(y6[64:128, :, :], 0.0)
            nc.gpsimd.memset(ww6[64:128, :, :], 0.0)

            # ---------------- filler pieces ----------------
            # Emission-order safety: a consumer (scores/attnV) emitted before
            # its producer piece reads stale data.  Pieces mark what they
            # produced; consumers force-pop fillers until the marker exists.
            ready = set()

            # During the lead-in (and tail) the sc/o psum banks are idle;
            # rotating filler psum through them keeps PE dense instead of
            # serializing every piece through the single kv bank.
            ps_rotate = {"on": False, "i": 0}

            def next_ps(name):
                if not ps_rotate["on"]:
                    return kv_pool.tile([128, 512], F32, tag="kv", name=name)
                i = ps_rotate["i"] = ps_rotate["i"] + 1
                k = i % 3
                if k == 0:
                    return kv_pool.tile([128, 512], F32, tag="kv", name=name)
                if k == 1:
                    sc = sc_pool.tile([128, SCW, 512], F32, tag="sc",
                                      name=name)
                    return sc[:, 0, :]
                return o_pool.tile([128, 512], F32, tag="o", name=name)

            xb_tiles = {}

            def piece_dma_block(n):
                def go():
                    xb = xs.tile([128, KO, 512], BF, tag="xb")
                    xb_tiles[n] = xb
                    nc.sync.dma_start(xb[:], xT_r[:, :, n * 512:(n + 1) * 512])
                    # fp8 pad cols (65:128) + ones col (64) for this block's
                    # j-tiles; gpsimd = off the critical engines
                    nc.gpsimd.memset(V8[:, :, 4 * n:4 * n + 4, 64:128], 0.0)
                    nc.gpsimd.memset(V8[:, :, 4 * n:4 * n + 4, 64:65], 1.0)
                return go

            def piece_k(n, p):
                def go():
                    xb = xb_tiles[n]
                    ps = next_ps(f"psk{n}_{p}")
                    for ko in range(KO):
                        nc.tensor.matmul(
                            ps[:], wk_sb[:, ko, p * 128:(p + 1) * 128],
                            xb[:, ko, :],
                            start=(ko == 0), stop=(ko == KO - 1),
                        )
                    nc.vector.tensor_scalar_add(
                        KT[:, p, n * 512:(n + 1) * 512], ps[:],
                        bk_sb[:, p:p + 1],
                    )
                    ready.add(("K", n, p))
                return go

            def piece_v(n, j4):
                def go():
                    xb = xb_tiles[n]
                    ps = next_ps(f"psv{n}_{j4}")
                    for ko in range(KO):
                        nc.tensor.matmul(
                            ps[:, :DH],
                            xb[:, ko, j4 * 128:(j4 + 1) * 128],
                            wv_sb[:, ko, :],
                            start=(ko == 0), stop=(ko == KO - 1),
                        )
                    nc.vector.tensor_copy(
                        V8[:, :, 4 * n + j4, 0:64],
                        ps[:, 0:DH].rearrange("l (h c) -> l h c", c=64),
                    )
                    ready.add(("V", 4 * n + j4))
                return go

            def block_pieces(n):
                ps = [piece_dma_block(n)]
                for p in range(NP):
                    ps.append(piece_k(n, p))
                for j4 in range(4):
                    ps.append(piece_v(n, j4))
                return ps

            ob_tiles = {}

            def piece_op(qh, m, n0, nw, heads, stage):
                # out-projection for m-tile cols [n0,n0+nw) over `heads`.
                # stage "full": all heads -> ob -> dma
                # stage "A": partial -> out_acc;  "B": rest + out_acc -> dma
                def go():
                    ms = slice(m * 128, (m + 1) * 128)
                    ps = next_ps(f"op{m}_{n0}_{stage}")
                    for i, h in enumerate(heads):
                        nc.tensor.matmul(
                            ps[:, :nw],
                            y6[:, h, ms],
                            ww6[:, h, n0:n0 + nw],
                            start=(i == 0), stop=(i == len(heads) - 1),
                        )
                    if stage == "A":
                        nc.vector.tensor_copy(
                            out_acc[:, m, n0:n0 + nw], ps[:, :nw])
                        return
                    if n0 == 0:
                        ob_tiles[m] = ob_pool.tile(
                            [128, D], F32, tag="ob", name=f"ob{m}")
                    ob = ob_tiles[m]
                    if stage == "B":
                        nc.vector.tensor_add(
                            ob[:, n0:n0 + nw], ps[:, :nw],
                            out_acc[:, m, n0:n0 + nw])
                    else:
                        nc.vector.tensor_copy(ob[:, n0:n0 + nw], ps[:, :nw])
                    if n0 + nw == D:
                        nc.sync.dma_start(out[ms, :], ob[:])
                return go

            xq_tiles = {}

            def piece_qproj_dma(nq):
                def go():
                    xqb = xs.tile([128, KO, 512], BF, tag="xb",
                                  name=f"xqb{nq}")
                    xq_tiles[nq] = xqb
                    nc.sync.dma_start(
                        xqb[:], xqT_r[:, :, nq * 512:(nq + 1) * 512])
                return go

            def piece_qproj_p(nq, p):
                def go():
                    xqb = xq_tiles[nq]
                    nqs = slice(nq * 512, (nq + 1) * 512)
                    psq = next_ps(f"psq{nq}_{p}")
                    for ko in range(KO):
                        nc.tensor.matmul(
                            psq[:], wq_sb[:, ko, p * 128:(p + 1) * 128],
                            xqb[:, ko, :],
                            start=(ko == 0), stop=(ko == KO - 1),
                        )
                    nc.vector.tensor_scalar_add(
                        QTz[0:64, 2 * p, nqs], psq[0:64, :],
                        bq_sb[0:64, p:p + 1],
                    )
                    nc.vector.tensor_scalar_add(
                        QTz[64:128, 2 * p + 1, nqs], psq[64:128, :],
                        bq_sb[64:128, p:p + 1],
                    )
                    ready.add(("Q", nq, p))
                return go

            # approximate PE cost (cycles at 2.4GHz) per filler piece kind
            COST_DMA = 200
            COST_K = KO * 512        # 3072
            COST_V = KO * 384        # 2304
            COST_QP = KO * 512       # 3072

            # ---------------- lead-in ----------------
            ps_rotate["on"] = True
            pieces0 = block_pieces(0)
            pieces0[0]()                        # xb0 DMA first in queue
            nc.sync.dma_start(wv_sb[:], wvT_r[:])
            nc.sync.dma_start(wq_sb[:], wqT_r[:])
            nc.sync.dma_start(bq_sb[:], bq[:])
            for piece in pieces0[1:]:
                piece()
            piece_qproj_dma(0)()
            piece_qproj_p(0, 0)()               # round 0 only needs p0
            piece_dma_block(1)()
            for p in range(NP):
                piece_k(1, p)()                 # K of block 1 (j4-7 scores)

            # ---------------- fused attention quarters ----------------
            ps_rotate["on"] = False
            for q in range(NSPLIT):
                if q == 0:
                    # priority: Q for upcoming p-rounds, V of block 1 (attnV
                    # pairs 2-3 of early rounds), then qh=1 Q + blocks 2,3
                    fillers = deque([
                        (piece_qproj_p(0, 1), COST_QP, 0),
                        (piece_v(1, 0), COST_V, 0), (piece_v(1, 1), COST_V, 0),
                        (piece_qproj_p(0, 2), COST_QP, 0),
                        (piece_v(1, 2), COST_V, 0), (piece_v(1, 3), COST_V, 0),
                        (piece_qproj_dma(1), COST_DMA, 0),
                        (piece_qproj_p(1, 0), COST_QP, 0),
                        (piece_qproj_p(1, 1), COST_QP, 0),
                        (piece_qproj_p(1, 2), COST_QP, 0),
                    ])
                    for n in (2, 3):
                        fillers.append((piece_dma_block(n), COST_DMA, 0))
                        for p in range(NP):
                            fillers.append((piece_k(n, p), COST_K, 0))
                        for j4 in range(4):
                            fillers.append((piece_v(n, j4), COST_V, 0))
                elif q < NSPLIT - 1:
                    if q == 1:
                        nc.sync.dma_start(ww6[0:64, :, :], ww6_r[:])
                    fillers = deque()
                    for n in (2 * q + 2, 2 * q + 3):
                        fillers.append((piece_dma_block(n), COST_DMA, 0))
                        for p in range(NP):
                            fillers.append((piece_k(n, p), COST_K, 0))
                        for j4 in range(4):
                            fillers.append((piece_v(n, j4), COST_V, 0))
                else:
                    # qh0 out-proj (6 heads) once qh0 is normalized (r6);
                    # qh1 heads 0-3 partials once its p0/p1 rounds are done
                    # (r10); qh1 heads 4,5 + out_acc finish in the tail
                    fillers = deque(
                        [(piece_op(0, m, n0, nw, range(NH), "full"),
                          KO * nw, 6)
                         for m in range(4)
                         for (n0, nw) in ((0, 512), (512, 256))]
                        + [(piece_op(1, m, n0, nw, range(4), "A"),
                            4 * nw, 10)
                           for m in range(4, 8)
                           for (n0, nw) in ((0, 512), (512, 256))]
                    )
                total_cost = sum(c for _, c, _ in fillers)
                n_slots = 12 * len(QCHUNKS)
                budget_rate = total_cost / n_slots
                budget = 0.0

                def require(marker):
                    # force-emit fillers until the producer of `marker` has
                    # been emitted (program order = dependency order)
                    while marker not in ready and fillers:
                        piece = fillers.popleft()[0]
                        piece()
                    assert marker in ready, f"missing producer {marker}"

                for r, (qh, p, h) in enumerate(ROUNDS):
                    h6 = 2 * p + h
                    qs = slice(qh * 512, (qh + 1) * 512)
                    ring = ptr[:, r % 2, :, :]
                    require(("Q", qh, p))
                    o_ps = o_pool.tile([128, 512], F32, tag="o")
                    pair_done = 0
                    jc = 0
                    for c, cs in enumerate(QCHUNKS):
                        for t in range(cs):
                            require(("K", (q * JQ + jc + t) // 4, p))
                        sc = sc_pool.tile([128, SCW, 512], F32, tag="sc")
                        nsc = 256 if PROBE == "halfscores" else 512
                        for t in range(cs):
                            j = q * JQ + jc + t
                            nc.tensor.matmul(
                                sc[:, t, :nsc],
                                KT[:, p, j * 128:(j + 1) * 128],
                                QTz[:, h6, qs][:, :nsc],
                                start=True, stop=True,
                            )
                        slot = jc % 6
                        nex = 256 if PROBE == "halfexp" else 512
                        nc.scalar.activation(
                            ring[:, slot:slot + cs, :nex], sc[:, :cs, :nex],
                            AF.Exp, scale=SCALE, bias=lnb[:],
                        )
                        jc_prev = jc
                        jc += cs
                        # attnV for pairs complete as of the PREVIOUS chunk:
                        # consuming this chunk's exp here would stall the
                        # in-order PE queue on the ACT engine every chunk
                        while 2 * (pair_done + 1) <= jc_prev:
                            t_l = pair_done
                            jg = q * JQ + 2 * t_l
                            require(("V", jg))
                            require(("V", jg + 1))
                            nc.tensor.matmul(
                                o_ps[:],
                                V8[:, h6, jg:jg + 2, :],
                                ring[:, (2 * t_l) % 6:(2 * t_l) % 6 + 2, :],
                                start=(t_l == 0), stop=(t_l == NPAIR - 1),
                                perf_mode=DR,
                            )
                            pair_done += 1
                        budget += budget_rate
                        while fillers and budget >= fillers[0][1] \
                                and fillers[0][2] <= r:
                            piece, cost, _ = fillers.popleft()
                            piece()
                            budget -= cost
                    while 2 * (pair_done + 1) <= jc:
                        t_l = pair_done
                        jg = q * JQ + 2 * t_l
                        require(("V", jg))
                        require(("V", jg + 1))
                        nc.tensor.matmul(
                            o_ps[:],
                            V8[:, h6, jg:jg + 2, :],
                            ring[:, (2 * t_l) % 6:(2 * t_l) % 6 + 2, :],
                            start=(t_l == 0), stop=(t_l == NPAIR - 1),
                            perf_mode=DR,
                        )
                        pair_done += 1
                    # ---- spill / accumulate / normalize ----
                    if q == 0:
                        nc.vector.tensor_copy(acc[0:65, r, :], o_ps[0:65, :])
                    else:
                        nc.vector.tensor_add(
                            acc[0:65, r, :], o_ps[0:65, :], acc[0:65, r, :])
                    if q == NSPLIT - 1:
                        dn = bc_pool.tile([1, 512], F32, tag="dn")
                        nc.vector.tensor_copy(dn[:], acc[64:65, r, :])
                        bc = bc_pool.tile([64, 512], F32, tag="bc")
                        nc.gpsimd.partition_broadcast(bc[:], dn[:], channels=64)
                        nc.vector.reciprocal(bc[:], bc[:])
                        nc.vector.tensor_mul(
                            y6[0:64, h6, qs], acc[0:64, r, :], bc[:])

                while fillers:
                    fillers.popleft()[0]()

            # ---------------- tail: qh1 heads 4,5 + out_acc ----------------
            ps_rotate["on"] = True
            for m in range(4, 8):
                for (n0, nw) in ((0, 512), (512, 256)):
                    piece_op(1, m, n0, nw, (4, 5), "B")()

            if debug:
                dKT = nc.dram_tensor("dKT", [128, NP, S], BF, kind="ExternalOutput")
                dQT = nc.dram_tensor("dQT", [128, NH, SQ], BF, kind="ExternalOutput")
                dV8 = nc.dram_tensor("dV8", [128, NH, NJ, 128], F8, kind="ExternalOutput")
                dacc = nc.dram_tensor("dacc", [128, 12, 512], F32, kind="ExternalOutput")
                dy6 = nc.dram_tensor("dy6", [128, NH, SQ], BF, kind="ExternalOutput")
                nc.sync.dma_start(dKT[:], KT[:])
                nc.sync.dma_start(dQT[:], QTz[:])
                nc.sync.dma_start(dV8[:], V8[:])
                nc.sync.dma_start(dacc[:], acc[:])
                nc.sync.dma_start(dy6[:], y6[:])

    nc.finalize()
    return nc


_NC_CACHE = None


def make_in_maps(x, wq, bq, wk, bk, wv, ww):
    x = np.ascontiguousarray(np.asarray(x, dtype=np.float32))
    xT_full = np.ascontiguousarray(x[0].T).astype(ml_dtypes.bfloat16)  # [D, S]
    in_maps = []
    for core in range(8):
        g, c = core // NC, core % NC
        gs = slice(g * DH, (g + 1) * DH)
        in_maps.append({
            "xT": xT_full,
            "xqT": np.ascontiguousarray(xT_full[:, c * SQ:(c + 1) * SQ]),
            "wqT": np.ascontiguousarray(wq[gs, :].T).astype(ml_dtypes.bfloat16),
            "wkT": np.ascontiguousarray(wk[gs, :].T).astype(ml_dtypes.bfloat16),
            "wvT": np.ascontiguousarray(wv[gs, :].T * VSCALE).astype(ml_dtypes.bfloat16),
            "wwT": np.ascontiguousarray(ww[:, gs].T / VSCALE).astype(ml_dtypes.bfloat16),
            "bq": np.ascontiguousarray(bq[gs].reshape(NP, 128).T).astype(np.float32),
            "bk": np.ascontiguousarray(bk[gs].reshape(NP, 128).T).astype(np.float32),
        })
    return in_maps


def kernel(x, wq, bq, wk, bk, wv, bv, ww, bw):
    global _NC_CACHE
    if _NC_CACHE is None:
        _NC_CACHE = build_nc()
    nc = _NC_CACHE

    in_maps = make_in_maps(x, wq, bq, wk, bk, wv, ww)
    res = run_bass_kernel_spmd(nc, in_maps, core_ids=list(range(8)))

    const_row = (bv @ ww.T + bw).astype(np.float32)  # [768]
    out = np.empty((1, S, D), dtype=np.float32)
    for c in range(NC):
        acc_out = res.results[0 * NC + c]["out"] + res.results[1 * NC + c]["out"]
        out[0, c * SQ:(c + 1) * SQ, :] = acc_out + const_row
    return out


# revision 38
# speedup vs baseline: 1.0258x; 1.0258x over previous
"""Multi-head attention (B=1, S=4096, D=768, H=12, Hd=64) on 8 trn2 cores.

Sharding: 2 head-groups (6 heads = 384 dims, Megatron column-split wq/wk/wv,
row-split ww) x 4 query-chunks (1024 rows).  core = g*4 + c.
Each core returns a partial output [1024, 768]; host sums the 2 group
partials per chunk and adds (bv @ ww.T + bw).

v2 design (ACT-bound fused schedule):
  - All projections / scores in bf16 (1 cyc/row on PE, exact-enough).
  - attnV in fp8e4 DoubleRow (2 key-tiles per instruction): V8 holds
    fp8(64*V) rows per key with a ones column at 64 and 63 pad cols (dual-fp8
    ldweights requires the full 128 weight columns); pt = fp8(8*exp(s/8))
    written directly by the ACT exp.  Scale bookkeeping: numerator rows are
    512*(P.V), denominator row 64 is 8*sum(P) -> y6 = 64*out; ww is
    pre-divided by 64 on the host.
  - The exp stream on ACT (164us of columns) is the binding engine.  The
    key axis is split in NSPLIT=4 quarters; within each quarter the 12
    rounds (qh, p, head) run scores->exp->attnV pipelined through a
    double-buffered [128,3,512] psum pair, while K/V projection blocks
    (quarters 0-2) and the out-projection (quarter 3) execute in the PE
    gaps as interleaved "filler" pieces.  attnV accumulates per-quarter in
    a single psum bank and spills/accumulates into acc (SBUF, f32).
  - psum budget: scores 2x3 banks + o 1 + filler 1 = 8.
"""

import sys

if "/opt/trn_rl_repo" not in sys.path:
    sys.path.insert(0, "/opt/trn_rl_repo")

import math
from collections import deque

import numpy as np
import ml_dtypes

import concourse.bacc as bacc
import concourse.bass as bass
import concourse.mybir as mybir
import concourse.tile as tile
from concourse.bass_utils import run_bass_kernel_spmd
from concourse.vector_clock import ScopedClock

F32 = mybir.dt.float32
BF = mybir.dt.bfloat16
F8 = mybir.dt.float8e4
AF = mybir.ActivationFunctionType
DR = mybir.MatmulPerfMode.DoubleRow

S = 4096          # sequence length
D = 768           # model dim
NG = 2            # head groups (cores axis 1)
NC = 4            # query chunks (cores axis 2)
DH = D // NG      # dims per group = 384
NP = DH // 128    # head pairs per group = 3
NH = 2 * NP       # heads per group = 6
SQ = S // NC      # queries per core = 1024
KO = D // 128     # contraction subtiles = 6
NJ = S // 128     # key tiles = 32
SCALE = 0.125     # 1/sqrt(64)
LN8 = float(math.log(8.0))
VSCALE = 32.0     # host scale folded into wv (and 1/VSCALE into ww);
                  # max |VSCALE*v| ~ 127 stays below the TRN e4m3 max of 240
                  # (the DVE f32->fp8 conversion overflows instead of
                  # saturating, so headroom is required)

import os
PROBE = os.environ.get("PROBE", "")   # timing probes: halfexp / halfscores

NSPLIT = 4        # key-axis quarters
JQ = NJ // NSPLIT           # j-tiles per quarter = 8
NPAIR = JQ // 2             # DoubleRow pairs per round-quarter = 4
if os.environ.get("CS_CFG", "2") == "2":
    QCHUNKS = [2, 2, 2, 2]  # exp chunk sizes covering JQ j-tiles
    SC_BUFS = 3             # psum: sc 3x2 banks + o + kv = 8
else:
    QCHUNKS = [3, 3, 2]
    SC_BUFS = 2             # psum: sc 2x3 banks + o + kv = 8
SCW = max(QCHUNKS)
ROUNDS = [(qh, p, h) for qh in range(2) for p in range(NP) for h in range(2)]

_PATCHED = False


def _patch_drain():
    """walrus in this container rejects >1 sync-wait per instruction
    ("Too many sync wait commands").  TileContext's tail drain aggregates one
    wait per live tile semaphore; redistribute them one-per-nop.  (Bacc's
    generate_event_semaphores handles the rest of the kernel.)"""
    global _PATCHED
    if _PATCHED:
        return
    _PATCHED = True

    def _drain_and_barrier(self, tick_clock, wait_clock):
        nc = self.nc
        drain_inst = nc.sync.drain()
        wait_clock.add_sem_waits(
            drain_inst.ins, ScopedClock({None: tick_clock.global_clock})
        )
        si = drain_inst.ins.sync_info
        waits = list(si.on_wait) if si is not None else []
        if len(waits) > 1:
            drain_inst.ins.sync_info = mybir.SyncInfo(
                on_wait=[waits[0]], on_update=list(si.on_update)
            )
            for w in waits[1:]:
                nop = nc.sync.nop(nofuse=True)
                nop.ins.sync_info = mybir.SyncInfo(on_wait=[w], on_update=[])
        nc.all_engine_barrier()
        assert self.sems is not None
        popped = nc._tile_sem_poison_stack.pop()
        assert popped is self._sem_poison
        nc.clear_and_free_semaphores(list(self.sems.allocated().values()))
        nc.all_engine_barrier()

    tile.TileContext._drain_and_barrier = _drain_and_barrier


def build_nc(loop_n=None, debug=False):
    _patch_drain()
    nc = bacc.Bacc("TRN2", target_bir_lowering=False)

    xT = nc.dram_tensor("xT", [D, S], BF, kind="ExternalInput")
    xqT = nc.dram_tensor("xqT", [D, SQ], BF, kind="ExternalInput")
    wqT = nc.dram_tensor("wqT", [D, DH], BF, kind="ExternalInput")
    wkT = nc.dram_tensor("wkT", [D, DH], BF, kind="ExternalInput")
    wvT = nc.dram_tensor("wvT", [D, DH], BF, kind="ExternalInput")  # x VSCALE
    wwT = nc.dram_tensor("wwT", [DH, D], BF, kind="ExternalInput")  # / VSCALE
    bq = nc.dram_tensor("bq", [128, NP], F32, kind="ExternalInput")
    bk = nc.dram_tensor("bk", [128, NP], F32, kind="ExternalInput")
    out = nc.dram_tensor("out", [SQ, D], F32, kind="ExternalOutput")

    xT_r = xT.rearrange("(ko p) n -> p ko n", p=128)
    xqT_r = xqT.rearrange("(ko p) n -> p ko n", p=128)
    wqT_r = wqT.rearrange("(ko p) m -> p ko m", p=128)
    wkT_r = wkT.rearrange("(ko p) m -> p ko m", p=128)
    wvT_r = wvT.rearrange("(ko p) m -> p ko m", p=128)
    ww6_r = wwT.rearrange("(h l) o -> l h o", l=64)   # [64, 6, 768]

    with tile.TileContext(nc) as tc:
        import contextlib

        with contextlib.ExitStack() as ctx:
            if loop_n is not None:
                ctx.enter_context(tc.For_i(0, loop_n, 1))
            persist = ctx.enter_context(tc.tile_pool(name="persist", bufs=1))
            KT = persist.tile([128, NP, S], BF)         # 24KB/part
            QTz = persist.tile([128, NH, SQ], BF)       # 12KB/part
            V8 = persist.tile([128, NH, NJ, 128], F8)   # 24KB/part
            acc = persist.tile([128, 12, 512], F32)     # 24KB/part
            ptr = persist.tile([128, 2, 6, 512], F8)    # exp rings, 6KB/part
            y6 = persist.tile([128, NH, SQ], BF)        # 12KB/part
            out_acc = persist.tile([128, 8, D], F32)    # 24KB/part
            ww6 = persist.tile([128, NH, D], BF)        # 9KB/part
            lnb = persist.tile([128, 1], F32)
            bq_sb = persist.tile([128, NP], F32)
            bk_sb = persist.tile([128, NP], F32)

            w_pool = ctx.enter_context(tc.tile_pool(name="w", bufs=1))
            wk_sb = w_pool.tile([128, KO, DH], BF)
            wv_sb = w_pool.tile([128, KO, DH], BF)
            wq_sb = w_pool.tile([128, KO, DH], BF)

            xs = ctx.enter_context(tc.tile_pool(name="xs", bufs=3))
            ob_pool = ctx.enter_context(tc.tile_pool(name="ob", bufs=2))
            bc_pool = ctx.enter_context(tc.tile_pool(name="bc", bufs=2))

            sc_pool = ctx.enter_context(
                tc.tile_pool(name="sc", bufs=SC_BUFS, space="PSUM"))
            o_pool = ctx.enter_context(
                tc.tile_pool(name="o", bufs=1, space="PSUM"))
            kv_pool = ctx.enter_context(
                tc.tile_pool(name="kv", bufs=1, space="PSUM"))

            # ---------------- init + weight DMA ----------------
            # DMA order matters: block-0 x data and wk must land first.
            nc.sync.dma_start(wk_sb[:], wkT_r[:])
            nc.sync.dma_start(bk_sb[:], bk[:])
            nc.vector.memset(lnb[:], LN8)
            # zero inits run on gpsimd (idle early) so the DVE queue stays
            # clear for the K bias-adds of the first blocks
            for h in range(NH):
                if h % 2 == 0:
                    nc.gpsimd.memset(QTz[64:128, h, :], 0.0)
                else:
                    nc.gpsimd.memset(QTz[0:64, h, :], 0.0)
            # upper halves zero: out-proj runs K=128 over head pairs' rows
            nc.gpsimd.memset(y6[64:128, :, :], 0.0)
            nc.gpsimd.memset(ww6[64:128, :, :], 0.0)

            # ---------------- filler pieces ----------------
            # Emission-order safety: a consumer (scores/attnV) emitted before
            # its producer piece reads stale data.  Pieces mark what they
            # produced; consumers force-pop fillers until the marker exists.
            ready = set()

            # During the lead-in (and tail) the sc/o psum banks are idle;
            # rotating filler psum through them keeps PE dense instead of
            # serializing every piece through the single kv bank.
            ps_rotate = {"on": False, "i": 0}

            def next_ps(name):
                if not ps_rotate["on"]:
                    return kv_pool.tile([128, 512], F32, tag="kv", name=name)
                i = ps_rotate["i"] = ps_rotate["i"] + 1
                k = i % 3
                if k == 0:
                    return kv_pool.tile([128, 512], F32, tag="kv", name=name)
                if k == 1:
                    sc = sc_pool.tile([128, SCW, 512], F32, tag="sc",
                                      name=name)
                    return sc[:, 0, :]
                return o_pool.tile([128, 512], F32, tag="o", name=name)

            xb_tiles = {}

            def piece_dma_block(n):
                def go():
                    xb = xs.tile([128, KO, 512], BF, tag="xb")
                    xb_tiles[n] = xb
                    nc.sync.dma_start(xb[:], xT_r[:, :, n * 512:(n + 1) * 512])
                    # fp8 pad cols (65:128) + ones col (64) for this block's
                    # j-tiles; gpsimd = off the critical engines
                    nc.gpsimd.memset(V8[:, :, 4 * n:4 * n + 4, 64:128], 0.0)
                    nc.gpsimd.memset(V8[:, :, 4 * n:4 * n + 4, 64:65], 1.0)
                return go

            def piece_k(n, p):
                def go():
                    xb = xb_tiles[n]
                    ps = next_ps(f"psk{n}_{p}")
                    for ko in range(KO):
                        nc.tensor.matmul(
                            ps[:], wk_sb[:, ko, p * 128:(p + 1) * 128],
                            xb[:, ko, :],
                            start=(ko == 0), stop=(ko == KO - 1),
                        )
                    nc.vector.tensor_scalar_add(
                        KT[:, p, n * 512:(n + 1) * 512], ps[:],
                        bk_sb[:, p:p + 1],
                    )
                    ready.add(("K", n, p))
                return go

            def piece_v(n, j4):
                def go():
                    xb = xb_tiles[n]
                    ps = next_ps(f"psv{n}_{j4}")
                    for ko in range(KO):
                        nc.tensor.matmul(
                            ps[:, :DH],
                            xb[:, ko, j4 * 128:(j4 + 1) * 128],
                            wv_sb[:, ko, :],
                            start=(ko == 0), stop=(ko == KO - 1),
                        )
                    nc.vector.tensor_copy(
                        V8[:, :, 4 * n + j4, 0:64],
                        ps[:, 0:DH].rearrange("l (h c) -> l h c", c=64),
                    )
                    ready.add(("V", 4 * n + j4))
                return go

            def block_pieces(n):
                ps = [piece_dma_block(n)]
                for p in range(NP):
                    ps.append(piece_k(n, p))
                for j4 in range(4):
                    ps.append(piece_v(n, j4))
                return ps

            ob_tiles = {}

            def piece_op(qh, m, n0, nw, heads, stage):
                # out-projection for m-tile cols [n0,n0+nw) over `heads`.
                # stage "full": all heads -> ob -> dma
                # stage "A": partial -> out_acc;  "B": rest + out_acc -> dma
                def go():
                    ms = slice(m * 128, (m + 1) * 128)
                    ps = next_ps(f"op{m}_{n0}_{stage}")
                    for i, h in enumerate(heads):
                        nc.tensor.matmul(
                            ps[:, :nw],
                            y6[:, h, ms],
                            ww6[:, h, n0:n0 + nw],
                            start=(i == 0), stop=(i == len(heads) - 1),
                        )
                    if stage == "A":
                        nc.vector.tensor_copy(
                            out_acc[:, m, n0:n0 + nw], ps[:, :nw])
                        return
                    if n0 == 0:
                        ob_tiles[m] = ob_pool.tile(
                            [128, D], F32, tag="ob", name=f"ob{m}")
                    ob = ob_tiles[m]
                    if stage == "B":
                        nc.vector.tensor_add(
                            ob[:, n0:n0 + nw], ps[:, :nw],
                            out_acc[:, m, n0:n0 + nw])
                    else:
                        nc.vector.tensor_copy(ob[:, n0:n0 + nw], ps[:, :nw])
                    if n0 + nw == D:
                        nc.sync.dma_start(out[ms, :], ob[:])
                return go

            xq_tiles = {}

            def piece_qproj_dma(nq):
                def go():
                    xqb = xs.tile([128, KO, 512], BF, tag="xb",
                                  name=f"xqb{nq}")
                    xq_tiles[nq] = xqb
                    nc.sync.dma_start(
                        xqb[:], xqT_r[:, :, nq * 512:(nq + 1) * 512])
                return go

            def piece_qproj_p(nq, p):
                def go():
                    xqb = xq_tiles[nq]
                    nqs = slice(nq * 512, (nq + 1) * 512)
                    psq = next_ps(f"psq{nq}_{p}")
                    for ko in range(KO):
                        nc.tensor.matmul(
                            psq[:], wq_sb[:, ko, p * 128:(p + 1) * 128],
                            xqb[:, ko, :],
                            start=(ko == 0), stop=(ko == KO - 1),
                        )
                    nc.vector.tensor_scalar_add(
                        QTz[0:64, 2 * p, nqs], psq[0:64, :],
                        bq_sb[0:64, p:p + 1],
                    )
                    nc.vector.tensor_scalar_add(
                        QTz[64:128, 2 * p + 1, nqs], psq[64:128, :],
                        bq_sb[64:128, p:p + 1],
                    )
                    ready.add(("Q", nq, p))
                return go

            # approximate PE cost (cycles at 2.4GHz) per filler piece kind
            COST_DMA = 200
            COST_K = KO * 512        # 3072
            COST_V = KO * 384        # 2304
            COST_QP = KO * 512       # 3072

            # ---------------- lead-in ----------------
            ps_rotate["on"] = True
            pieces0 = block_pieces(0)
            pieces0[0]()                        # xb0 DMA first in queue
            nc.sync.dma_start(wv_sb[:], wvT_r[:])
            nc.sync.dma_start(wq_sb[:], wqT_r[:])
            nc.sync.dma_start(bq_sb[:], bq[:])
            for piece in pieces0[1:]:
                piece()
            piece_qproj_dma(0)()
            piece_qproj_p(0, 0)()               # round 0 only needs p0
            piece_dma_block(1)()
            for p in range(NP):
                piece_k(1, p)()                 # K of block 1 (j4-7 scores)

            # ---------------- fused attention quarters ----------------
            ps_rotate["on"] = False
            for q in range(NSPLIT):
                if q == 0:
                    # priority: Q for upcoming p-rounds, V of block 1 (attnV
                    # pairs 2-3 of early rounds), then qh=1 Q + blocks 2,3
                    fillers = deque([
                        (piece_qproj_p(0, 1), COST_QP, 0),
                        (piece_v(1, 0), COST_V, 0), (piece_v(1, 1), COST_V, 0),
                        (piece_qproj_p(0, 2), COST_QP, 0),
                        (piece_v(1, 2), COST_V, 0), (piece_v(1, 3), COST_V, 0),
                        (piece_qproj_dma(1), COST_DMA, 0),
                        (piece_qproj_p(1, 0), COST_QP, 0),
                        (piece_qproj_p(1, 1), COST_QP, 0),
                        (piece_qproj_p(1, 2), COST_QP, 0),
                    ])
                    for n in (2, 3):
                        fillers.append((piece_dma_block(n), COST_DMA, 0))
                        for p in range(NP):
                            fillers.append((piece_k(n, p), COST_K, 0))
                        for j4 in range(4):
                            fillers.append((piece_v(n, j4), COST_V, 0))
                elif q < NSPLIT - 1:
                    if q == 1:
                        nc.sync.dma_start(ww6[0:64, :, :], ww6_r[:])
                    fillers = deque()
                    for n in (2 * q + 2, 2 * q + 3):
                        fillers.append((piece_dma_block(n), COST_DMA, 0))
                        for p in range(NP):
                            fillers.append((piece_k(n, p), COST_K, 0))
                        for j4 in range(4):
                            fillers.append((piece_v(n, j4), COST_V, 0))
                else:
                    # qh0 out-proj (6 heads) once qh0 is normalized (r6);
                    # qh1 heads 0-3 partials once its p0/p1 rounds are done
                    # (r10); qh1 heads 4,5 + out_acc finish in the tail
                    fillers = deque(
                        [(piece_op(0, m, n0, nw, range(NH), "full"),
                          KO * nw, 6)
                         for m in range(4)
                         for (n0, nw) in ((0, 512), (512, 256))]
                        + [(piece_op(1, m, n0, nw, range(4), "A"),
                            4 * nw, 10)
                           for m in range(4, 8)
                           for (n0, nw) in ((0, 512), (512, 256))]
                    )
                total_cost = sum(c for _, c, _ in fillers)
                n_slots = 12 * len(QCHUNKS)
                budget_rate = total_cost / n_slots
                budget = 0.0

                def require(marker):
                    # force-emit fillers until the producer of `marker` has
                    # been emitted (program order = dependency order)
                    while marker not in ready and fillers:
                        piece = fillers.popleft()[0]
                        piece()
                    assert marker in ready, f"missing producer {marker}"

                for r, (qh, p, h) in enumerate(ROUNDS):
                    h6 = 2 * p + h
                    qs = slice(qh * 512, (qh + 1) * 512)
                    ring = ptr[:, r % 2, :, :]
                    require(("Q", qh, p))
                    o_ps = o_pool.tile([128, 512], F32, tag="o")
                    pair_done = 0
                    jc = 0
                    for c, cs in enumerate(QCHUNKS):
                        for t in range(cs):
                            require(("K", (q * JQ + jc + t) // 4, p))
                        sc = sc_pool.tile([128, SCW, 512], F32, tag="sc")
                        nsc = 256 if PROBE == "halfscores" else 512
                        for t in range(cs):
                            j = q * JQ + jc + t
                            nc.tensor.matmul(
                                sc[:, t, :nsc],
                                KT[:, p, j * 128:(j + 1) * 128],
                                QTz[:, h6, qs][:, :nsc],
                                start=True, stop=True,
                            )
                        slot = jc % 6
                        nex = 256 if PROBE == "halfexp" else 512
                        nc.scalar.activation(
                            ring[:, slot:slot + cs, :nex], sc[:, :cs, :nex],
                            AF.Exp, scale=SCALE, bias=lnb[:],
                        )
                        jc_prev = jc
                        jc += cs
                        # attnV for pairs complete as of the PREVIOUS chunk:
                        # consuming this chunk's exp here would stall the
                        # in-order PE queue on the ACT engine every chunk
                        while 2 * (pair_done + 1) <= jc_prev:
                            t_l = pair_done
                            jg = q * JQ + 2 * t_l
                            require(("V", jg))
                            require(("V", jg + 1))
                            nc.tensor.matmul(
                                o_ps[:],
                                V8[:, h6, jg:jg + 2, :],
                                ring[:, (2 * t_l) % 6:(2 * t_l) % 6 + 2, :],
                                start=(t_l == 0), stop=(t_l == NPAIR - 1),
                                perf_mode=DR,
                            )
                            pair_done += 1
                        if fillers and fillers[0][2] <= r:
                            budget += budget_rate
                            npop = 0
                            while fillers and budget >= fillers[0][1] \
                                    and fillers[0][2] <= r and npop < 2:
                                piece, cost, _ = fillers.popleft()
                                piece()
                                budget -= cost
                                npop += 1
                        else:
                            budget = min(budget + budget_rate, 2 * 3072)
                    while 2 * (pair_done + 1) <= jc:
                        t_l = pair_done
                        jg = q * JQ + 2 * t_l
                        require(("V", jg))
                        require(("V", jg + 1))
                        nc.tensor.matmul(
                            o_ps[:],
                            V8[:, h6, jg:jg + 2, :],
                            ring[:, (2 * t_l) % 6:(2 * t_l) % 6 + 2, :],
                            start=(t_l == 0), stop=(t_l == NPAIR - 1),
                            perf_mode=DR,
                        )
                        pair_done += 1
                    # ---- spill / accumulate / normalize ----
                    if q == 0:
                        nc.vector.tensor_copy(acc[0:65, r, :], o_ps[0:65, :])
                    else:
                        nc.vector.tensor_add(
                            acc[0:65, r, :], o_ps[0:65, :], acc[0:65, r, :])
                    if q == NSPLIT - 1:
                        dn = bc_pool.tile([1, 512], F32, tag="dn")
                        nc.vector.tensor_copy(dn[:], acc[64:65, r, :])
                        bc = bc_pool.tile([64, 512], F32, tag="bc")
                        nc.gpsimd.partition_broadcast(bc[:], dn[:], channels=64)
                        nc.vector.reciprocal(bc[:], bc[:])
                        nc.vector.tensor_mul(
                            y6[0:64, h6, qs], acc[0:64, r, :], bc[:])

                while fillers:
                    fillers.popleft()[0]()

            # ---------------- tail: qh1 heads 4,5 + out_acc ----------------
            ps_rotate["on"] = True
            for m in range(4, 8):
                for (n0, nw) in ((0, 512), (512, 256)):
                    piece_op(1, m, n0, nw, (4, 5), "B")()

            if debug:
                dKT = nc.dram_tensor("dKT", [128, NP, S], BF, kind="ExternalOutput")
                dQT = nc.dram_tensor("dQT", [128, NH, SQ], BF, kind="ExternalOutput")
                dV8 = nc.dram_tensor("dV8", [128, NH, NJ, 128], F8, kind="ExternalOutput")
                dacc = nc.dram_tensor("dacc", [128, 12, 512], F32, kind="ExternalOutput")
                dy6 = nc.dram_tensor("dy6", [128, NH, SQ], BF, kind="ExternalOutput")
                nc.sync.dma_start(dKT[:], KT[:])
                nc.sync.dma_start(dQT[:], QTz[:])
                nc.sync.dma_start(dV8[:], V8[:])
                nc.sync.dma_start(dacc[:], acc[:])
                nc.sync.dma_start(dy6[:], y6[:])

    nc.finalize()
    return nc


_NC_CACHE = None


def make_in_maps(x, wq, bq, wk, bk, wv, ww):
    x = np.ascontiguousarray(np.asarray(x, dtype=np.float32))
    xT_full = np.ascontiguousarray(x[0].T).astype(ml_dtypes.bfloat16)  # [D, S]
    in_maps = []
    for core in range(8):
        g, c = core // NC, core % NC
        gs = slice(g * DH, (g + 1) * DH)
        in_maps.append({
            "xT": xT_full,
            "xqT": np.ascontiguousarray(xT_full[:, c * SQ:(c + 1) * SQ]),
            "wqT": np.ascontiguousarray(wq[gs, :].T).astype(ml_dtypes.bfloat16),
            "wkT": np.ascontiguousarray(wk[gs, :].T).astype(ml_dtypes.bfloat16),
            "wvT": np.ascontiguousarray(wv[gs, :].T * VSCALE).astype(ml_dtypes.bfloat16),
            "wwT": np.ascontiguousarray(ww[:, gs].T / VSCALE).astype(ml_dtypes.bfloat16),
            "bq": np.ascontiguousarray(bq[gs].reshape(NP, 128).T).astype(np.float32),
            "bk": np.ascontiguousarray(bk[gs].reshape(NP, 128).T).astype(np.float32),
        })
    return in_maps


def kernel(x, wq, bq, wk, bk, wv, bv, ww, bw):
    global _NC_CACHE
    if _NC_CACHE is None:
        _NC_CACHE = build_nc()
    nc = _NC_CACHE

    in_maps = make_in_maps(x, wq, bq, wk, bk, wv, ww)
    res = run_bass_kernel_spmd(nc, in_maps, core_ids=list(range(8)))

    const_row = (bv @ ww.T + bw).astype(np.float32)  # [768]
    out = np.empty((1, S, D), dtype=np.float32)
    for c in range(NC):
        acc_out = res.results[0 * NC + c]["out"] + res.results[1 * NC + c]["out"]
        out[0, c * SQ:(c + 1) * SQ, :] = acc_out + const_row
    return out
